# revision 1
# baseline (speedup 1.0000x reference)
"""Multi-head attention (B=2, N=2048, D=1024, H=16, d_k=d_v=64) on 8
TRN2 NeuronCores.

Sharding: data parallel over batch (2) x tensor parallel over head
groups (4 heads per core). Each core computes its 4 heads' attention
plus the partial output projection (Wp rows for those heads); the host
sums the 4 partials per batch and adds the residual.

Per-core kernel layout choices:
  - scores are computed TRANSPOSED (S^T: keys on partitions, queries on
    free dim) so that attn@v needs no transposes: lhsT = v (natural
    [seq, d_v] layout), rhs = exp(S^T).
  - softmax skips max-subtraction (scores are ~N(0,1)*8*0.125 -> |s|<8,
    exp is safe in fp32) and the key-axis sum comes for free from a
    ones-column appended to v (M=65 matmul).
  - matmuls run in float32r (full PE rate at N>=256, ~tf32 precision);
    exp output / attn weights are bf16.
"""
import numpy as np

import concourse.bass as bass
import concourse.tile as tile
from concourse import mybir
from concourse.vector_clock import ScopedClock

f32 = mybir.dt.float32
f32r = mybir.dt.float32r
bf16 = mybir.dt.bfloat16

B, N, D = 2, 2048, 1024
H, DK = 16, 64
HPC = 4          # heads per core
GCOLS = HPC * DK  # 256 weight columns per core
NCORES = 8
P = 128
NKB = N // P      # 16 key blocks
NQB = N // 512    # 4 query blocks of 512
NDMC = D // P     # 8 d_model chunks
NSB = N // P      # 16 seq blocks for the projection

_cache = {}
_last_results = None


# ---------------------------------------------------------------------------
# Workarounds for this walrus build: max ONE sync wait per instruction.
# ---------------------------------------------------------------------------
_ws_counter = [0]


def _split_multi_waits(nc, limit=1):
    for f in nc.m.functions:
        for bb in f.blocks:
            new = []
            changed = False
            for inst in bb.instructions:
                si = inst.sync_info
                waits = list(si.on_wait) if si is not None and si.on_wait else []
                if len(waits) > limit:
                    changed = True
                    extra = waits[:-limit]
                    for i in range(0, len(extra), limit):
                        _ws_counter[0] += 1
                        nop = mybir.InstNoOp(
                            name=f"I-waitsplit-{_ws_counter[0]}", ins=[], outs=[]
                        )
                        nop.engine = inst.engine
                        nop.sync_info = mybir.SyncInfo(
                            on_wait=extra[i : i + limit], on_update=[]
                        )
                        new.append(nop)
                    si.on_wait = waits[-limit:]
                    inst.sync_info = si
                new.append(inst)
            if changed:
                bb.instructions = new


def _patched_drain_and_barrier(self, tick_clock, wait_clock):
    nc = self.nc
    drain_inst = nc.sync.drain()
    wait_clock.add_sem_waits(
        drain_inst.ins, ScopedClock({None: tick_clock.global_clock})
    )
    si = drain_inst.ins.sync_info
    if si is not None and si.on_wait is not None and len(si.on_wait) > 1:
        waits = list(si.on_wait)
        si.on_wait = waits[:1]
        drain_inst.ins.sync_info = si
        for i in range(1, len(waits)):
            extra = nc.sync.drain()
            esi = extra.ins.sync_info
            if esi is None:
                esi = mybir.SyncInfo(on_wait=[], on_update=[])
            esi.on_wait = waits[i : i + 1]
            extra.ins.sync_info = esi
    nc.all_engine_barrier()
    assert self.sems is not None
    popped = nc._tile_sem_poison_stack.pop()
    assert popped is self._sem_poison
    nc.clear_and_free_semaphores(list(self.sems.allocated().values()))
    nc.all_engine_barrier()


tile.TileContext._drain_and_barrier = _patched_drain_and_barrier


# ---------------------------------------------------------------------------
# Kernel build
# ---------------------------------------------------------------------------
def _build():
    nc = bass.Bass()
    xT = nc.dram_tensor("xT", [D, N], bf16, kind="ExternalInput")
    wq = nc.dram_tensor("wq", [D, GCOLS], bf16, kind="ExternalInput")
    wk = nc.dram_tensor("wk", [D, GCOLS], bf16, kind="ExternalInput")
    wv = nc.dram_tensor("wv", [D, GCOLS], bf16, kind="ExternalInput")
    wp = nc.dram_tensor("wp", [GCOLS, D], bf16, kind="ExternalInput")
    ones = nc.dram_tensor("ones", [1, P], f32r, kind="ExternalInput")
    pout = nc.dram_tensor("pout", [N, D], bf16, kind="ExternalOutput")

    with tile.TileContext(nc) as tc:
        import contextlib

        with contextlib.ExitStack() as ctx:
            sbX = ctx.enter_context(tc.tile_pool(name="sbX", bufs=1))
            sbW = ctx.enter_context(tc.tile_pool(name="sbW", bufs=1))
            sbQK = ctx.enter_context(tc.tile_pool(name="sbQK", bufs=1))
            sbV = ctx.enter_context(tc.tile_pool(name="sbV", bufs=1))
            sbO = ctx.enter_context(tc.tile_pool(name="sbO", bufs=1))
            sbA = ctx.enter_context(tc.tile_pool(name="sbA", bufs=4))
            sbR = ctx.enter_context(tc.tile_pool(name="sbR", bufs=3))
            sbP = ctx.enter_context(tc.tile_pool(name="sbP", bufs=3))
            drS = ctx.enter_context(tc.tile_pool(name="drS", bufs=2,
                                                  space="DRAM"))
            psA = ctx.enter_context(tc.tile_pool(name="psA", bufs=2, space="PSUM"))
            psS = ctx.enter_context(tc.tile_pool(name="psS", bufs=2, space="PSUM"))
            psO = ctx.enter_context(tc.tile_pool(name="psO", bufs=2, space="PSUM"))

            # ---- loads ----------------------------------------------------
            ones_sb = sbW.tile([1, P], f32r, tag="ones")
            nc.sync.dma_start(out=ones_sb[:], in_=ones[:])

            xt = []
            for i in range(NDMC):
                t = sbX.tile([P, N], bf16, tag=f"xt{i}", name=f"xt{i}")
                nc.sync.dma_start(out=t[:], in_=xT[i * P : (i + 1) * P, :])
                xt.append(t)

            wq_t, wk_t, wv_t = [], [], []
            for wname, dram, lst in (("wq", wq, wq_t), ("wk", wk, wk_t),
                                     ("wv", wv, wv_t)):
                for i in range(NDMC):
                    t = sbW.tile([P, GCOLS], bf16, tag=f"{wname}{i}",
                                 name=f"{wname}{i}")
                    nc.sync.dma_start(out=t[:], in_=dram[i * P : (i + 1) * P, :])
                    lst.append(t)

            wp_t = []
            for g2 in range(2):
                t = sbW.tile([P, D], bf16, tag=f"wp{g2}", name=f"wpt{g2}")
                nc.sync.dma_start(out=t[:], in_=wp[g2 * P : (g2 + 1) * P, :])
                wp_t.append(t)

            # Pre-warm the exp table (~2.7us ACT table load) during loads.
            warm = sbR.tile([1, 2], f32, tag="warm")
            nc.scalar.activation(
                warm[:], ones_sb[0:1, 0:2], mybir.ActivationFunctionType.Exp
            )

            qT = [sbQK.tile([P, N], bf16, tag=f"qT{g2}", name=f"qT{g2}")
                  for g2 in range(2)]
            kT = [sbQK.tile([P, N], bf16, tag=f"kT{g2}", name=f"kT{g2}")
                  for g2 in range(2)]
            vaug = [sbV.tile([P, NKB, 2, 65], bf16, tag=f"vaug{g2}",
                             name=f"vaug{g2}") for g2 in range(2)]
            for g2 in range(2):
                nc.vector.memset(vaug[g2][:, :, :, 64:65], 1.0)
            outT = [sbO.tile([P, N], bf16, tag=f"outT{g2}", name=f"outT{g2}")
                    for g2 in range(2)]

            # ---- chain thunk builders (for side-work interleaving) --------
            def qk_chain_thunks(dst, w_t, g2, qb):
                st = {}
                def start():
                    st["p"] = psA.tile([P, 512], f32, tag="pacc",
                                       name=f"pqk{g2}_{qb}_{id(w_t)%97}")
                    nc.tensor.matmul(
                        st["p"][:], w_t[0][:, g2 * P : (g2 + 1) * P],
                        xt[0][:, qb * 512 : (qb + 1) * 512],
                        start=True, stop=False,
                    )
                def mid(c):
                    nc.tensor.matmul(
                        st["p"][:], w_t[c][:, g2 * P : (g2 + 1) * P],
                        xt[c][:, qb * 512 : (qb + 1) * 512],
                        start=False, stop=(c == NDMC - 1),
                    )
                def evict():
                    nc.vector.tensor_copy(
                        dst[g2][:, qb * 512 : (qb + 1) * 512], st["p"][:]
                    )
                return [start] + [lambda c=c: mid(c) for c in range(1, NDMC)] + [evict]

            def v_chain_thunks(kb):
                st = {}
                def start():
                    st["p"] = psA.tile([P, 2, P], f32, tag="pacc",
                                       name=f"pv{kb}")
                    nc.tensor.matmul(
                        st["p"][:], xt[0][:, kb * P : (kb + 1) * P], wv_t[0][:],
                        start=True, stop=False,
                    )
                def mid(c):
                    nc.tensor.matmul(
                        st["p"][:], xt[c][:, kb * P : (kb + 1) * P], wv_t[c][:],
                        start=False, stop=(c == NDMC - 1),
                    )
                def ev(g2):
                    nc.vector.tensor_copy(
                        vaug[g2][:, kb, :, 0:64], st["p"][:, g2, :]
                    )
                return ([start] + [lambda c=c: mid(c) for c in range(1, NDMC)]
                        + [lambda: ev(0), lambda: ev(1)])

            def proj_thunks(sb):
                st = {}
                ot = sbP.tile([P, D], bf16, tag="pout", name=f"ot{sb}")
                def mk(half):
                    def start():
                        st[half] = psA.tile([P, 512], f32, tag="pacc",
                                            name=f"pp{sb}_{half}")
                        nc.tensor.matmul(
                            st[half][:], outT[0][:, sb * P : (sb + 1) * P],
                            wp_t[0][:, half * 512 : (half + 1) * 512],
                            start=True, stop=False,
                        )
                    def second():
                        nc.tensor.matmul(
                            st[half][:], outT[1][:, sb * P : (sb + 1) * P],
                            wp_t[1][:, half * 512 : (half + 1) * 512],
                            start=False, stop=True,
                        )
                    def evict():
                        nc.vector.tensor_copy(
                            ot[:, half * 512 : (half + 1) * 512], st[half][:]
                        )
                    return [start, second, evict]
                def dma():
                    nc.sync.dma_start(
                        out=pout[sb * P : (sb + 1) * P, :], in_=ot[:]
                    )
                return mk(0) + mk(1) + [dma]

            side = []          # queue of pending side-work thunks

            def pull_side(slots_left):
                if not side:
                    return
                n = 1
                if slots_left > 0 and len(side) > slots_left:
                    n = -(-len(side) // slots_left)
                for _ in range(min(n, len(side))):
                    side.pop(0)()

            state = {"pv": None, "pmul": None}

            def emit_norm(g2, qb, po):
                # Stage A: free the po PSUM slots fast (copy to SBUF),
                # reciprocal, and launch the DRAM-round-trip broadcast.
                # Stage B (the multiplies) is deferred a couple of slots so
                # the in-order DVE queue never blocks on the DMA latency.
                pend = []
                for h in range(2):
                    oc = sbR.tile([65, 512], f32, tag="ocopy",
                                  name=f"oc{g2}_{qb}_{h}")
                    nc.vector.tensor_copy(oc[:], po[h][:])
                    rc = sbR.tile([1, 512], f32, tag="recip",
                                  name=f"rc{g2}_{qb}_{h}")
                    nc.vector.reciprocal(rc[:], oc[64:65, :])
                    rcd = drS.tile([1, 512], f32, tag="rcd",
                                   name=f"rcd{g2}_{qb}_{h}")
                    nc.sync.dma_start(out=rcd[:], in_=rc[:])
                    bc = sbR.tile([64, 512], f32, tag="bcast",
                                  name=f"bc{g2}_{qb}_{h}")
                    nc.sync.dma_start(out=bc[:],
                                      in_=rcd[:].partition_broadcast(64))
                    pend.append((h, oc, bc))
                def muls():
                    for h, oc, bc in pend:
                        nc.vector.tensor_mul(
                            outT[g2][h * 64 : (h + 1) * 64,
                                     qb * 512 : (qb + 1) * 512],
                            oc[0:64, :],
                            bc[:],
                        )
                state["pmul"] = muls

            def flush_pending():
                if state["pv"] is None:
                    return
                pg2, pqb, pkb, ppo, pat = state["pv"]
                for h in range(2):
                    nc.tensor.matmul(
                        ppo[h][:],
                        vaug[pg2][:, pkb, h, :],
                        pat[:, h, :],
                        start=(pkb == 0), stop=(pkb == NKB - 1),
                    )
                if pkb == NKB - 1:
                    emit_norm(pg2, pqb, ppo)
                state["pv"] = None

            def emit_attention(g2, fill_hook, side_enabled, slots_after):
                for qb in range(NQB):
                    po = [psO.tile([65, 512], f32, tag="o",
                                   name=f"po{g2}_{qb}_{h}") for h in range(2)]
                    for kb in range(NKB):
                        if kb == 2 and state["pmul"] is not None:
                            state["pmul"]()
                            state["pmul"] = None
                        ps = psS.tile([P, 2, 512], f32, tag="s",
                                      name=f"ps{g2}_{qb}_{kb}")
                        at = sbA.tile([P, 2, 512], bf16, tag="attnT",
                                      name=f"at{g2}_{qb}_{kb}")
                        for h in range(2):
                            nc.tensor.matmul(
                                ps[:, h, :],
                                kT[g2][h * 64 : (h + 1) * 64,
                                       kb * P : (kb + 1) * P],
                                qT[g2][h * 64 : (h + 1) * 64,
                                       qb * 512 : (qb + 1) * 512],
                                start=True, stop=True,
                                tile_position=(h * 64, 0),
                            )
                        nc.scalar.activation(
                            at[:], ps[:], mybir.ActivationFunctionType.Exp,
                            scale=0.125,
                        )
                        fill_hook(qb, kb)
                        flush_pending()
                        state["pv"] = (g2, qb, kb, po, at)
                        if side_enabled(qb):
                            slots_left = (NQB - 1 - qb) * NKB \
                                + (NKB - 1 - kb) + slots_after
                            pull_side(slots_left)

            def emit_chain(thunks):
                for t in thunks:
                    t()

            # ---- emission schedule ---------------------------------------
            # minimal prefix, q/k chains interleaved chunk-wise so they
            # track the xT DMA arrival instead of running serially after.
            qch = qk_chain_thunks(qT, wq_t, 0, 0)
            kch = qk_chain_thunks(kT, wk_t, 0, 0)
            for a, b in zip(qch, kch):
                a()
                b()
            emit_chain(v_chain_thunks(0))

            # pair-1 q/k chains ride in attention-0's slack (qb >= 1)
            side.extend(qk_chain_thunks(qT, wq_t, 1, 0))
            for b in range(NQB):
                side.extend(qk_chain_thunks(kT, wk_t, 1, b))
            for b in range(1, NQB):
                side.extend(qk_chain_thunks(qT, wq_t, 1, b))

            # fill queue entries: (kind, idx, is_last, thunk). Drains are
            # REQUIREMENT-driven: before slot kb+1's scores/attnv can be
            # emitted, v chain kb+1 and k chain (kb+1)//4 must be fully
            # emitted (emission order defines Tile's dependency direction).
            fillq = []
            fill_state = {"v": 0, "k": 0}

            def fq_push(kind, idx, thunks):
                for i, t in enumerate(thunks):
                    fillq.append((kind, idx, i == len(thunks) - 1, t))

            def fq_pop():
                kind, idx, last, t = fillq.pop(0)
                t()
                if last and kind in fill_state:
                    fill_state[kind] = idx

            def fill0(qb, kb):
                if qb == 0:
                    if kb == 0:
                        order = [("v", 1), ("v", 2), ("k", 1), ("v", 3),
                                 ("v", 4), ("v", 5), ("k", 2), ("v", 6),
                                 ("v", 7), ("v", 8), ("v", 9), ("k", 3),
                                 ("v", 10), ("v", 11), ("v", 12), ("v", 13),
                                 ("v", 14), ("v", 15)]
                        for kind, j in order:
                            if kind == "v":
                                fq_push("v", j, v_chain_thunks(j))
                            else:
                                fq_push("k", j,
                                        qk_chain_thunks(kT, wk_t, 0, j))
                    needv = min(kb + 2, NKB - 1)
                    needk = min((kb + 1) // 4, 3)
                    while fillq and (fill_state["v"] < needv
                                     or fill_state["k"] < needk):
                        fq_pop()
                    for _ in range(min(len(fillq), 4)):
                        fq_pop()
                if kb == 13 and qb < NQB - 1:
                    emit_chain(qk_chain_thunks(qT, wq_t, 0, qb + 1))

            emit_attention(0, fill0, lambda qb: qb >= 1,
                           slots_after=8)
            while side:
                side.pop(0)()

            # attention pair 1: projection rides in the slack, gated on norm
            proj_release = {(1, 4): [0], (1, 8): [1], (1, 12): [2],
                            (2, 1): [3], (2, 4): [4], (2, 8): [5],
                            (2, 12): [6],
                            (3, 1): [7], (3, 4): [8], (3, 8): [9],
                            (3, 12): [10]}

            def fill1(qb, kb):
                for sb in proj_release.get((qb, kb), []):
                    side.extend(proj_thunks(sb))

            emit_attention(1, fill1, lambda qb: True, slots_after=0)
            flush_pending()
            if state["pmul"] is not None:
                state["pmul"]()
                state["pmul"] = None
            for sb in [11, 12, 13, 14, 15]:
                side.extend(proj_thunks(sb))
            while side:
                side.pop(0)()

    _split_multi_waits(nc)
    return nc


def make_in_maps(x, Wq, Wk, Wv, Wp):
    import ml_dtypes

    bf = ml_dtypes.bfloat16
    x = np.ascontiguousarray(x, dtype=np.float32)
    Wq = np.asarray(Wq, dtype=np.float32)
    Wk = np.asarray(Wk, dtype=np.float32)
    Wv = np.asarray(Wv, dtype=np.float32)
    Wp = np.asarray(Wp, dtype=np.float32)
    ones_np = np.ones((1, P), dtype=np.float32)
    in_maps = []
    for c in range(NCORES):
        b, g = divmod(c, 4)
        cs = slice(g * GCOLS, (g + 1) * GCOLS)
        in_maps.append(
            {
                "xT": np.ascontiguousarray(x[b].T).astype(bf),
                "wq": np.ascontiguousarray(Wq[:, cs]).astype(bf),
                "wk": np.ascontiguousarray(Wk[:, cs]).astype(bf),
                "wv": np.ascontiguousarray(Wv[:, cs]).astype(bf),
                "wp": np.ascontiguousarray(Wp[cs, :]).astype(bf),
                "ones": ones_np,
            }
        )
    return in_maps


def kernel(x, Wq, Wk, Wv, Wp):
    global _last_results
    from concourse.bass_utils import run_bass_kernel_spmd

    x = np.ascontiguousarray(x, dtype=np.float32)

    if "nc" not in _cache:
        _cache["nc"] = _build()
    nc = _cache["nc"]

    in_maps = make_in_maps(x, Wq, Wk, Wv, Wp)
    res = run_bass_kernel_spmd(nc, in_maps, core_ids=list(range(NCORES)))
    _last_results = res

    out = np.empty((B, N, D), dtype=np.float32)
    for b in range(B):
        acc = x[b].copy()
        for g in range(4):
            acc += res.results[b * 4 + g]["pout"].astype(np.float32)
        out[b] = acc
    return out



# revision 6
# speedup vs baseline: 1.1168x; 1.1168x over previous
"""Multi-head attention (B=2, N=2048, D=1024, H=16, d_k=d_v=64) on 8
TRN2 NeuronCores.

Sharding: data parallel over batch (2) x tensor parallel over head
groups (4 heads per core). Each core computes its 4 heads' attention
plus the partial output projection (Wp rows for those heads); the host
sums the 4 partials per batch and adds the residual.

v2 design notes (vs the 257us baseline):
  - exp is split between ScalarE (true exp) and VectorE (Schraudolph
    int16-bit-trick exp: bits = round(128*log2e*s + 16250.4) viewed as
    bf16, ~3% max rel err on the assigned slots) so ACT is no longer a
    ~143us serial bottleneck.
  - softmax reciprocal: denominators are staged through DRAM into a
    [128, 8] layout so one 128-lane reciprocal replaces the pathological
    [1,512] 1-lane reciprocals (53us -> ~1us).
  - loads: wqk packed into one tensor, x split column-wise, issued from
    BOTH sync and scalar DMA queues so the first score matmul starts
    ~10us earlier.
  - attention blocks alternate g2 (head-pair) per qb so projection and
    normalization work spreads evenly; proj matmuls ride in the slack.
"""
import numpy as np

import concourse.bass as bass
import concourse.tile as tile
from concourse import mybir
from concourse.vector_clock import ScopedClock

f32 = mybir.dt.float32
f32r = mybir.dt.float32r
bf16 = mybir.dt.bfloat16
i16 = mybir.dt.int16

B, N, D = 2, 2048, 1024
H, DK = 16, 64
HPC = 4          # heads per core
GCOLS = HPC * DK  # 256 weight columns per core
NCORES = 8
P = 128
NKB = N // P      # 16 key blocks
NQB = N // 512    # 4 query blocks of 512
NDMC = D // P     # 8 d_model chunks

# Schraudolph fast-exp constants (DVE int16 trick), calibrated on HW:
# bits = round_i16(EXP_A * (q.k) + EXP_B); bits viewed as bf16 ~= exp(s/8)
EXP_A = 0.125 * 1.4426950408889634 * 128.0   # 23.0831...
EXP_B = 16256.0 - 5.6
# slots (by kb) whose exp runs on DVE instead of ACT. ~6.5/16 of slots.
DVE_KBS_EVEN = frozenset({4, 6, 8, 10, 12, 14})
DVE_KBS_ODD = frozenset({4, 5, 8, 10, 12, 14})

_cache = {}
_last_results = None


# ---------------------------------------------------------------------------
# Workarounds for this walrus build: max ONE sync wait per instruction.
# ---------------------------------------------------------------------------
_ws_counter = [0]


def _split_multi_waits(nc, limit=1):
    for f in nc.m.functions:
        for bb in f.blocks:
            new = []
            changed = False
            for inst in bb.instructions:
                si = inst.sync_info
                waits = list(si.on_wait) if si is not None and si.on_wait else []
                if len(waits) > limit:
                    changed = True
                    extra = waits[:-limit]
                    for i in range(0, len(extra), limit):
                        _ws_counter[0] += 1
                        nop = mybir.InstNoOp(
                            name=f"I-waitsplit-{_ws_counter[0]}", ins=[], outs=[]
                        )
                        nop.engine = inst.engine
                        nop.sync_info = mybir.SyncInfo(
                            on_wait=extra[i : i + limit], on_update=[]
                        )
                        new.append(nop)
                    si.on_wait = waits[-limit:]
                    inst.sync_info = si
                new.append(inst)
            if changed:
                bb.instructions = new


def _patched_drain_and_barrier(self, tick_clock, wait_clock):
    nc = self.nc
    drain_inst = nc.sync.drain()
    wait_clock.add_sem_waits(
        drain_inst.ins, ScopedClock({None: tick_clock.global_clock})
    )
    si = drain_inst.ins.sync_info
    if si is not None and si.on_wait is not None and len(si.on_wait) > 1:
        waits = list(si.on_wait)
        si.on_wait = waits[:1]
        drain_inst.ins.sync_info = si
        for i in range(1, len(waits)):
            extra = nc.sync.drain()
            esi = extra.ins.sync_info
            if esi is None:
                esi = mybir.SyncInfo(on_wait=[], on_update=[])
            esi.on_wait = waits[i : i + 1]
            extra.ins.sync_info = esi
    nc.all_engine_barrier()
    assert self.sems is not None
    popped = nc._tile_sem_poison_stack.pop()
    assert popped is self._sem_poison
    nc.clear_and_free_semaphores(list(self.sems.allocated().values()))
    nc.all_engine_barrier()


tile.TileContext._drain_and_barrier = _patched_drain_and_barrier


# ---------------------------------------------------------------------------
# Kernel build
# ---------------------------------------------------------------------------
def _build():
    nc = bass.Bass()
    xT = nc.dram_tensor("xT", [D, N], bf16, kind="ExternalInput")
    wqk = nc.dram_tensor("wqk", [D, 2 * GCOLS], bf16, kind="ExternalInput")
    wv = nc.dram_tensor("wv", [D, GCOLS], bf16, kind="ExternalInput")
    wp = nc.dram_tensor("wp", [GCOLS, D], bf16, kind="ExternalInput")
    ones = nc.dram_tensor("ones", [1, P], f32r, kind="ExternalInput")
    pout = nc.dram_tensor("pout", [N, D], bf16, kind="ExternalOutput")

    with tile.TileContext(nc) as tc:
        import contextlib

        with contextlib.ExitStack() as ctx:
            sbX = ctx.enter_context(tc.tile_pool(name="sbX", bufs=1))
            sbW = ctx.enter_context(tc.tile_pool(name="sbW", bufs=1))
            sbQK = ctx.enter_context(tc.tile_pool(name="sbQK", bufs=1))
            sbV = ctx.enter_context(tc.tile_pool(name="sbV", bufs=1))
            sbO = ctx.enter_context(tc.tile_pool(name="sbO", bufs=1))
            sbA = ctx.enter_context(tc.tile_pool(name="sbA", bufs=4))
            sbR = ctx.enter_context(tc.tile_pool(name="sbR", bufs=3))
            sbP = ctx.enter_context(tc.tile_pool(name="sbP", bufs=3))
            drS = ctx.enter_context(tc.tile_pool(name="drS", bufs=4,
                                                  space="DRAM"))
            psS = ctx.enter_context(tc.tile_pool(name="psS", bufs=2, space="PSUM"))
            psO = ctx.enter_context(tc.tile_pool(name="psO", bufs=1, space="PSUM"))
            psA = ctx.enter_context(tc.tile_pool(name="psA", bufs=2, space="PSUM"))

            # ---- loads ----------------------------------------------------
            # sync engine: ones, wqk chunks, then xtB + wp.
            # scalar engine: xtA (first 512 cols) + wv interleaved.
            ones_sb = sbW.tile([1, P], f32r, tag="ones")
            nc.sync.dma_start(out=ones_sb[:], in_=ones[:])

            # Pre-warm the exp table (~2.7us ACT table load) first so the
            # scalar engine's load DMAs queue behind it.
            warm = sbR.tile([1, 2], f32, tag="warm")
            nc.scalar.activation(
                warm[:], ones_sb[0:1, 0:2], mybir.ActivationFunctionType.Exp
            )

            wqk_t = []
            for c in range(NDMC):
                t = sbW.tile([P, 2 * GCOLS], bf16, tag=f"wqk{c}",
                             name=f"wqk{c}")
                nc.sync.dma_start(out=t[:], in_=wqk[c * P : (c + 1) * P, :])
                wqk_t.append(t)

            xt = [sbX.tile([P, N], bf16, tag=f"xt{c}", name=f"xt{c}")
                  for c in range(NDMC)]
            wv_t = []
            for c in range(NDMC):
                nc.scalar.dma_start(out=xt[c][:, 0:512],
                                    in_=xT[c * P : (c + 1) * P, 0:512])
                t = sbW.tile([P, GCOLS], bf16, tag=f"wv{c}", name=f"wv{c}")
                nc.scalar.dma_start(out=t[:], in_=wv[c * P : (c + 1) * P, :])
                wv_t.append(t)

            # xtB in two column waves: cols 512:1024 are needed first (by
            # k(0,1), q(*,1) and v(4..7)); 1024:2048 later.
            for c in range(NDMC):
                nc.sync.dma_start(out=xt[c][:, 512:1024],
                                  in_=xT[c * P : (c + 1) * P, 512:1024])
            for c in range(NDMC):
                nc.sync.dma_start(out=xt[c][:, 1024:N],
                                  in_=xT[c * P : (c + 1) * P, 1024:N])

            wp_t = []
            for g2 in range(2):
                t = sbW.tile([P, D], bf16, tag=f"wp{g2}", name=f"wpt{g2}")
                nc.scalar.dma_start(out=t[:], in_=wp[g2 * P : (g2 + 1) * P, :])
                wp_t.append(t)

            qT = [sbQK.tile([P, N], bf16, tag=f"qT{g2}", name=f"qT{g2}")
                  for g2 in range(2)]
            kT = [sbQK.tile([P, N], bf16, tag=f"kT{g2}", name=f"kT{g2}")
                  for g2 in range(2)]
            vaug = [sbV.tile([P, NKB, 2, 65], bf16, tag=f"vaug{g2}",
                             name=f"vaug{g2}") for g2 in range(2)]
            for g2 in range(2):
                nc.vector.memset(vaug[g2][:, :, :, 64:65], 1.0)
            outT = [sbO.tile([P, N], bf16, tag=f"outT{g2}", name=f"outT{g2}")
                    for g2 in range(2)]

            # ---- chain thunk builders ------------------------------------
            def qk_chain_thunks(dst, which, g2, qb):
                # which: 0 = wq (cols 0:256), 1 = wk (cols 256:512)
                base = which * GCOLS + g2 * P
                st = {}
                def start():
                    st["p"] = psA.tile([P, 512], f32, tag="pacc",
                                       name=f"pqk{which}_{g2}_{qb}")
                    nc.tensor.matmul(
                        st["p"][:], wqk_t[0][:, base : base + P],
                        xt[0][:, qb * 512 : (qb + 1) * 512],
                        start=True, stop=False,
                    )
                def mid(c):
                    nc.tensor.matmul(
                        st["p"][:], wqk_t[c][:, base : base + P],
                        xt[c][:, qb * 512 : (qb + 1) * 512],
                        start=False, stop=(c == NDMC - 1),
                    )
                def evict():
                    nc.vector.tensor_copy(
                        dst[g2][:, qb * 512 : (qb + 1) * 512], st["p"][:]
                    )
                return [start] + [lambda c=c: mid(c) for c in range(1, NDMC)] + [evict]

            def v_chain_thunks(kb):
                st = {}
                def start():
                    st["p"] = psA.tile([P, 2, P], f32, tag="pacc",
                                       name=f"pv{kb}")
                    nc.tensor.matmul(
                        st["p"][:], xt[0][:, kb * P : (kb + 1) * P], wv_t[0][:],
                        start=True, stop=False,
                    )
                def mid(c):
                    nc.tensor.matmul(
                        st["p"][:], xt[c][:, kb * P : (kb + 1) * P], wv_t[c][:],
                        start=False, stop=(c == NDMC - 1),
                    )
                def ev(g2):
                    nc.vector.tensor_copy(
                        vaug[g2][:, kb, :, 0:64], st["p"][:, g2, :]
                    )
                return ([start] + [lambda c=c: mid(c) for c in range(1, NDMC)]
                        + [lambda: ev(0), lambda: ev(1)])

            def proj_thunks(sb):
                # per seq block sb: 2 psum halves; evictions split ACT/DVE.
                ot = sbP.tile([P, D], bf16, tag="pout", name=f"ot{sb}")
                st = {}
                def mk(half):
                    def start():
                        st[half] = psA.tile([P, 512], f32, tag="pacc",
                                            name=f"pp{sb}_{half}")
                        nc.tensor.matmul(
                            st[half][:], outT[0][:, sb * P : (sb + 1) * P],
                            wp_t[0][:, half * 512 : (half + 1) * 512],
                            start=True, stop=False,
                        )
                    def second():
                        nc.tensor.matmul(
                            st[half][:], outT[1][:, sb * P : (sb + 1) * P],
                            wp_t[1][:, half * 512 : (half + 1) * 512],
                            start=False, stop=True,
                        )
                    def evict():
                        dst = ot[:, half * 512 : (half + 1) * 512]
                        if (sb + half) % 2 == 0:
                            nc.scalar.copy(dst, st[half][:])
                        else:
                            nc.vector.tensor_copy(dst, st[half][:])
                    return [start, second, evict]
                def dma():
                    nc.sync.dma_start(
                        out=pout[sb * P : (sb + 1) * P, :], in_=ot[:]
                    )
                return mk(0) + mk(1) + [dma]

            # ---- side-work queue -----------------------------------------
            side = []           # list of (tag, thunk); tag may be None
            done_tags = set()

            def _pop_one():
                tag, t = side.pop(0)
                t()
                if tag is not None:
                    done_tags.add(tag)

            def ensure(tag):
                while tag not in done_tags:
                    assert side, f"dependency {tag} not in side queue"
                    _pop_one()

            def pull_side(slots_left):
                if not side:
                    return
                n = 1
                if slots_left > 0 and len(side) > slots_left:
                    n = -(-len(side) // slots_left)
                for _ in range(min(n, len(side))):
                    _pop_one()

            def push_chain(tag, thunks):
                for i, t in enumerate(thunks):
                    side.append((tag if i == len(thunks) - 1 else None, t))

            state = {"pv": None, "pmul": None}

            # ---- softmax normalization chain -----------------------------
            def emit_norm(g2, qb, po):
                # immediate: free the po psum (den + oc copies), launch the
                # DRAM round trips; deferred: the two outT multiplies.
                den = sbR.tile([1, 1024], f32, tag="den",
                               name=f"den{g2}_{qb}")
                nc.vector.tensor_copy(den[:], po[64:65, :])
                oc = sbR.tile([64, 1024], f32, tag="ocopy",
                              name=f"oc{g2}_{qb}")
                nc.vector.tensor_copy(oc[:], po[0:64, :])
                d1 = drS.tile([P, 8], f32, tag="d1", name=f"d1_{g2}_{qb}")
                nc.sync.dma_start(out=d1[:], in_=den[:])
                d128 = sbR.tile([P, 8], f32, tag="d128",
                                name=f"d128_{g2}_{qb}")
                nc.sync.dma_start(out=d128[:], in_=d1[:])
                rec = sbR.tile([P, 8], f32, tag="rec", name=f"rec{g2}_{qb}")
                nc.vector.reciprocal(rec[:], d128[:])
                d2 = drS.tile([1, 1024], f32, tag="d2", name=f"d2_{g2}_{qb}")
                nc.sync.dma_start(out=d2[:], in_=rec[:])
                bc = sbR.tile([64, 1024], f32, tag="bcast",
                              name=f"bc{g2}_{qb}")
                nc.sync.dma_start(out=bc[:], in_=d2[:].partition_broadcast(64))

                def muls():
                    for h in range(2):
                        nc.vector.tensor_mul(
                            outT[g2][h * 64 : (h + 1) * 64,
                                     qb * 512 : (qb + 1) * 512],
                            oc[:, h * 512 : (h + 1) * 512],
                            bc[:, h * 512 : (h + 1) * 512],
                        )
                state["pmul"] = muls

            def flush_pending():
                if state["pv"] is None:
                    return
                pg2, pqb, pkb, ppo, pat = state["pv"]
                for h in range(2):
                    nc.tensor.matmul(
                        ppo[:, h * 512 : (h + 1) * 512],
                        vaug[pg2][:, pkb, h, :],
                        pat[:, h * 512 : (h + 1) * 512],
                        start=(pkb == 0), stop=(pkb == NKB - 1),
                    )
                if pkb == NKB - 1:
                    emit_norm(pg2, pqb, ppo)
                state["pv"] = None

            # ---- one attention block (g2, qb): 16 kb slots ----------------
            def attention_block(g2, qb):
                dve_kbs = DVE_KBS_ODD if qb % 2 else DVE_KBS_EVEN
                po = psO.tile([65, 1024], f32, tag="o",
                              name=f"po{g2}_{qb}")
                for kb in range(NKB):
                    ensure(("k", g2, kb // 4))
                    ensure(("v", kb))
                    if kb == 2 and state["pmul"] is not None:
                        state["pmul"]()
                        state["pmul"] = None
                    # side work rides BEFORE the flush in early slots so the
                    # PE queue has independent work while po frees up.
                    slots_left = (NQB * 2 - 1 - (qb * 2 + g2)) * NKB \
                        + (NKB - 1 - kb)
                    if kb in (1, 2, 3):
                        pull_side(slots_left + 6)
                    ps = psS.tile([P, 1024], f32, tag="s",
                                  name=f"ps{g2}_{qb}_{kb}")
                    at = sbA.tile([P, 1024], bf16, tag="attnT",
                                  name=f"at{g2}_{qb}_{kb}")
                    for h in range(2):
                        nc.tensor.matmul(
                            ps[:, h * 512 : (h + 1) * 512],
                            kT[g2][h * 64 : (h + 1) * 64,
                                   kb * P : (kb + 1) * P],
                            qT[g2][h * 64 : (h + 1) * 64,
                                   qb * 512 : (qb + 1) * 512],
                            start=True, stop=True,
                            tile_position=(h * 64, 0),
                        )
                    if kb in dve_kbs:
                        nc.vector.tensor_scalar(
                            at[:].bitcast(i16), ps[:], EXP_A, EXP_B,
                            mybir.AluOpType.mult, mybir.AluOpType.add,
                        )
                    else:
                        nc.scalar.activation(
                            at[:], ps[:], mybir.ActivationFunctionType.Exp,
                            scale=0.125,
                        )
                    flush_pending()
                    state["pv"] = (g2, qb, kb, po, at)
                    if kb not in (1, 2, 3):
                        pull_side(slots_left)

            # ---- emission schedule ---------------------------------------
            # prefix: q(0,0) + k(0,0) chains interleaved chunk-wise (track
            # DMA arrival), then v(0).
            qch = qk_chain_thunks(qT, 0, 0, 0)
            kch = qk_chain_thunks(kT, 1, 0, 0)
            for a, b in zip(qch, kch):
                a()
                b()
            for t in v_chain_thunks(0):
                t()
            done_tags.add(("q", 0, 0))
            done_tags.add(("k", 0, 0))
            done_tags.add(("v", 0))

            # side queue, in dependency-need order for block (0,0) then
            # block (1,0), then the rest.
            for j in (1, 2):
                push_chain(("v", j), v_chain_thunks(j))
            push_chain(("k", 0, 1), qk_chain_thunks(kT, 1, 0, 1))
            for j in (3, 4, 5):
                push_chain(("v", j), v_chain_thunks(j))
            push_chain(("k", 0, 2), qk_chain_thunks(kT, 1, 0, 2))
            for j in (6, 7):
                push_chain(("v", j), v_chain_thunks(j))
            push_chain(("q", 1, 0), qk_chain_thunks(qT, 0, 1, 0))
            for j in (8, 9):
                push_chain(("v", j), v_chain_thunks(j))
            push_chain(("k", 0, 3), qk_chain_thunks(kT, 1, 0, 3))
            for j in (10, 11):
                push_chain(("v", j), v_chain_thunks(j))
            push_chain(("k", 1, 0), qk_chain_thunks(kT, 1, 1, 0))
            for j in (12, 13):
                push_chain(("v", j), v_chain_thunks(j))
            push_chain(("k", 1, 1), qk_chain_thunks(kT, 1, 1, 1))
            for j in (14, 15):
                push_chain(("v", j), v_chain_thunks(j))
            push_chain(("k", 1, 2), qk_chain_thunks(kT, 1, 1, 2))
            push_chain(("k", 1, 3), qk_chain_thunks(kT, 1, 1, 3))
            push_chain(("q", 0, 1), qk_chain_thunks(qT, 0, 0, 1))
            push_chain(("q", 1, 1), qk_chain_thunks(qT, 0, 1, 1))
            push_chain(("q", 0, 2), qk_chain_thunks(qT, 0, 0, 2))
            push_chain(("q", 1, 2), qk_chain_thunks(qT, 0, 1, 2))
            push_chain(("q", 0, 3), qk_chain_thunks(qT, 0, 0, 3))
            push_chain(("q", 1, 3), qk_chain_thunks(qT, 0, 1, 3))

            # attention blocks: qb-major, alternating g2.  The norm muls for
            # (1, qb-1) fire at slot 2 of block (0, qb); proj group qb-1 is
            # released right after that block so it rides in (1, qb)'s slack.
            for qb in range(NQB):
                for g2 in range(2):
                    if qb or g2:
                        ensure(("q", g2, qb))
                    attention_block(g2, qb)
                    if g2 == 0 and qb >= 1:
                        for sb in range(4 * (qb - 1), 4 * qb):
                            push_chain(None, proj_thunks(sb))

            flush_pending()
            if state["pmul"] is not None:
                state["pmul"]()
                state["pmul"] = None
            for sb in range(12, 16):
                push_chain(None, proj_thunks(sb))
            while side:
                _pop_one()

    _split_multi_waits(nc)
    return nc


def make_in_maps(x, Wq, Wk, Wv, Wp):
    import ml_dtypes

    bf = ml_dtypes.bfloat16
    x = np.ascontiguousarray(x, dtype=np.float32)
    Wq = np.asarray(Wq, dtype=np.float32)
    Wk = np.asarray(Wk, dtype=np.float32)
    Wv = np.asarray(Wv, dtype=np.float32)
    Wp = np.asarray(Wp, dtype=np.float32)
    ones_np = np.ones((1, P), dtype=np.float32)
    in_maps = []
    for c in range(NCORES):
        b, g = divmod(c, 4)
        cs = slice(g * GCOLS, (g + 1) * GCOLS)
        wqk = np.concatenate([Wq[:, cs], Wk[:, cs]], axis=1)
        in_maps.append(
            {
                "xT": np.ascontiguousarray(x[b].T).astype(bf),
                "wqk": np.ascontiguousarray(wqk).astype(bf),
                "wv": np.ascontiguousarray(Wv[:, cs]).astype(bf),
                "wp": np.ascontiguousarray(Wp[cs, :]).astype(bf),
                "ones": ones_np,
            }
        )
    return in_maps


def kernel(x, Wq, Wk, Wv, Wp):
    global _last_results
    from concourse.bass_utils import run_bass_kernel_spmd

    x = np.ascontiguousarray(x, dtype=np.float32)

    if "nc" not in _cache:
        _cache["nc"] = _build()
    nc = _cache["nc"]

    in_maps = make_in_maps(x, Wq, Wk, Wv, Wp)
    res = run_bass_kernel_spmd(nc, in_maps, core_ids=list(range(NCORES)))
    _last_results = res

    out = np.empty((B, N, D), dtype=np.float32)
    for b in range(B):
        acc = x[b].copy()
        for g in range(4):
            acc += res.results[b * 4 + g]["pout"].astype(np.float32)
        out[b] = acc
    return out
